# revision 3
# baseline (speedup 1.0000x reference)
"""Trainium2 Bass kernel: multi-head attention (B=4, S=2048, D=1024, H=16, HD=64).

Sharding: 8 cores = 4 batches x 2 head-groups. Core c handles batch c//2,
heads (c%2)*8 .. +8. Each core computes a partial output projection
out_partial[b] = ctx(heads) @ Wo[head_rows]; host sums the two partials per
batch and adds bo.

On-core layout ("k-major"): logits are computed transposed, LT[k, q], so the
softmax sum over keys is a partition-dim reduction done on the PE (fused into
the ctx matmul via an extra all-(mask)ones column appended to V), and the
attention-weighted sum ctxT[hd, q] = V'.T @ exp(LT) comes out in exactly the
layout the output projection needs as its stationary operand. No transposes of
the S x S matrix are ever needed. Softmax max-subtraction is skipped: logits
are ~N(0,1) here (X ~ N(0,1), W ~ N(0,1)/sqrt(D)), exp is safe in fp32, and
softmax is shift-invariant so the result matches the reference.

The additive -1e6 mask penalty is implemented exactly (for binary masks) by
zeroing masked keys' columns of V and the ones-column: exp(x - 1e6) underflows
to 0.0 in fp32 in the reference too, so weights and normalizer agree.

Matmuls run in float32r (TF32-like, 1 row/cycle at N>=512 vs fp32's 4): all
matmul-input tiles are float32r-typed so their producers round on write, which
the walrus verifier requires.
"""

import os
import sys

import numpy as np

sys.path.insert(0, "/opt/trn_rl_repo")

B, S, D = 4, 2048, 1024
H, HD = 16, 64
NCORES = 8
HPC = H // 2  # heads per core
CW = HPC * HD  # per-core head-channel width (512)
P = 128
NKT = S // P  # 16 key tiles of 128
USE_F32R = os.environ.get("KERNEL_F32R", "1") == "1"

_cache = {}


def _build():
    from concourse import bacc, masks, mybir, tile

    dt = mybir.dt
    f32 = dt.float32
    mdt = dt.float32r if USE_F32R else f32  # matmul-input dtype
    Exp = mybir.ActivationFunctionType.Exp
    mult = mybir.AluOpType.mult

    nc = bacc.Bacc("TRN2", debug=False, target_bir_lowering=False, num_devices=NCORES)

    X_d = nc.dram_tensor("X", [S, D], f32, kind="ExternalInput").ap()
    mask_d = nc.dram_tensor("mask", [S], f32, kind="ExternalInput").ap()
    Wq_d = nc.dram_tensor("Wq", [D, CW], mdt, kind="ExternalInput").ap()
    Wk_d = nc.dram_tensor("Wk", [D, CW], mdt, kind="ExternalInput").ap()
    Wv_d = nc.dram_tensor("Wv", [D, CW], mdt, kind="ExternalInput").ap()
    bq_d = nc.dram_tensor("bq", [CW], f32, kind="ExternalInput").ap()
    bk_d = nc.dram_tensor("bk", [CW], f32, kind="ExternalInput").ap()
    bv_d = nc.dram_tensor("bv", [CW], f32, kind="ExternalInput").ap()
    Wo_d = nc.dram_tensor("Wo", [CW, D], mdt, kind="ExternalInput").ap()
    out_d = nc.dram_tensor("out", [S, D], f32, kind="ExternalOutput").ap()

    with tile.TileContext(nc) as tc:
        with (
            tc.tile_pool(name="const", bufs=1) as cpool,
            tc.tile_pool(name="dst", bufs=1) as dstpool,
        ):
            ident = cpool.tile([P, P], f32, tag="ident")
            masks.make_identity(nc, ident[:])
            ones_f = cpool.tile([P, 64], f32, tag="ones_f")
            nc.gpsimd.memset(ones_f[:], 1.0)
            ones_t = cpool.tile([P, 64], mdt, tag="ones")
            nc.vector.tensor_copy(ones_t[:], ones_f[:])
            ones8 = cpool.tile([P, HPC, 1], f32, tag="ones8")
            nc.gpsimd.memset(ones8[:], 1.0)
            mask_t = cpool.tile([P, NKT], f32, tag="maskt")
            nc.gpsimd.dma_start(out=mask_t[:], in_=mask_d.rearrange("(kt i) -> i kt", i=P))
            bq_t = cpool.tile([P, 4], f32, tag="bqt")
            nc.gpsimd.dma_start(out=bq_t[:], in_=bq_d.rearrange("(p i) -> i p", i=P))
            bk_t = cpool.tile([P, 4], f32, tag="bkt")
            nc.gpsimd.dma_start(out=bk_t[:], in_=bk_d.rearrange("(p i) -> i p", i=P))
            bv_t = cpool.tile([HD, HPC], f32, tag="bvt")
            nc.gpsimd.dma_start(out=bv_t[:], in_=bv_d.rearrange("(h i) -> i h", i=HD))

            # QT/KT: [d-channel packs of 128 (2 heads), seq]; V': [k, kt, head, HD+1]
            QT = dstpool.tile([P, 4, S], mdt, tag="QT")
            KT = dstpool.tile([P, 4, S], mdt, tag="KT")
            Vt = dstpool.tile([P, NKT, HPC, HD + 1], mdt, tag="V")

            # ---- Phase 1+2: X transpose + Q/K/V projections (two seq-halves) ----
            with (
                tc.tile_pool(name="xtp", bufs=2) as xtpool,
                tc.tile_pool(name="tps", bufs=3, space="PSUM") as tpsum,
                tc.tile_pool(name="qps", bufs=3, space="PSUM") as qpsum,
            ):
                for half in range(2):
                    XTh = xtpool.tile([P, 8, S // 2], mdt, tag="xt", bufs=1)
                    for q8 in range(8):
                        qt = half * 8 + q8
                        xin = xtpool.tile([P, D], f32, tag="xin")
                        nc.sync.dma_start(out=xin[:], in_=X_d[qt * P : (qt + 1) * P, :])
                        for dc in range(8):
                            tp = tpsum.tile([P, P], f32, tag="tp")
                            nc.tensor.transpose(tp[:], xin[:, dc * P : (dc + 1) * P], ident[:])
                            nc.vector.tensor_copy(XTh[:, dc, q8 * P : (q8 + 1) * P], tp[:])
                    for W_d, b_t, dst in ((Wq_d, bq_t, QT), (Wk_d, bk_t, KT)):
                        Wre = W_d.rearrange("(dc p) m -> p dc m", p=P)
                        for pack in range(4):
                            wt = xtpool.tile([P, 8, P], mdt, tag="wqk")
                            nc.sync.dma_start(out=wt[:], in_=Wre[:, :, pack * P : (pack + 1) * P])
                            for q2 in range(2):
                                ps = qpsum.tile([P, 512], f32, tag="qp")
                                for dc in range(8):
                                    nc.tensor.matmul(
                                        ps[:],
                                        wt[:, dc, :],
                                        XTh[:, dc, q2 * 512 : (q2 + 1) * 512],
                                        start=(dc == 0),
                                        stop=(dc == 7),
                                    )
                                lo = half * 1024 + q2 * 512
                                nc.vector.tensor_scalar_add(
                                    dst[:, pack, lo : lo + 512], ps[:], b_t[:, pack : pack + 1]
                                )
                    vwt = xtpool.tile([P, 8, 512], mdt, tag="wv", bufs=1)
                    nc.sync.dma_start(out=vwt[:], in_=Wv_d.rearrange("(dc p) m -> p dc m", p=P))
                    for k8 in range(8):
                        kt = half * 8 + k8
                        ps = qpsum.tile([P, 512], f32, tag="qp")
                        for dc in range(8):
                            nc.tensor.matmul(
                                ps[:],
                                XTh[:, dc, k8 * P : (k8 + 1) * P],
                                vwt[:, dc, :],
                                start=(dc == 0),
                                stop=(dc == 7),
                            )
                        # masked V (bv folded into ctx later) + mask column for Z
                        nc.vector.tensor_scalar_mul(
                            Vt[:, kt, :, 0:HD],
                            ps.rearrange("p (h e) -> p h e", e=HD),
                            mask_t[:, kt : kt + 1],
                        )
                        nc.vector.tensor_scalar_mul(
                            Vt[:, kt, :, HD : HD + 1], ones8[:], mask_t[:, kt : kt + 1]
                        )

            # ---- Phase 3: attention per (head, 512-query tile) ----
            with tc.tile_pool(name="ctxp", bufs=1) as ctxpool:
                ctxn = ctxpool.tile([HD, HPC, S], mdt, tag="ctxn")
                with (
                    tc.tile_pool(name="attn", bufs=3) as apool,
                    tc.tile_pool(name="lps", bufs=3, space="PSUM") as lpsum,
                    tc.tile_pool(name="cps", bufs=2, space="PSUM") as cpsum,
                    tc.tile_pool(name="zps", bufs=2, space="PSUM") as zpsum,
                ):
                    for h in range(HPC):
                        hb = (h % 2) * 64
                        pk = h // 2
                        for q4 in range(4):
                            qs = slice(q4 * 512, (q4 + 1) * 512)
                            ctxps = cpsum.tile([HD + 1, 512], f32, tag="ctx")
                            ets = []
                            # software-pipelined: logits MM runs 2 ahead of ctx MM
                            for kc in range(NKT + 2):
                                if kc < NKT:
                                    lps = lpsum.tile([P, 512], f32, tag="lg")
                                    nc.tensor.matmul(
                                        lps[:],
                                        KT[hb : hb + 64, pk, kc * P : (kc + 1) * P],
                                        QT[hb : hb + 64, pk, qs],
                                        start=True,
                                        stop=True,
                                    )
                                    et = apool.tile([P, 512], mdt, tag="exp")
                                    nc.scalar.activation(et[:], lps[:], Exp, scale=0.125)
                                    ets.append(et)
                                if kc >= 2:
                                    kd = kc - 2
                                    nc.tensor.matmul(
                                        ctxps[:],
                                        Vt[:, kd, h, :],
                                        ets[kd][:],
                                        start=(kd == 0),
                                        stop=(kd == NKT - 1),
                                    )
                            zt = apool.tile([P, 512], mdt, tag="z")
                            with nc.allow_low_precision(reason="f32r feed; recip is exact f32"):
                                nc.vector.reciprocal(zt[64:65, :], ctxps[64:65, :])
                            zb = zpsum.tile([64, 512], f32, tag="zb")
                            nc.tensor.matmul(
                                zb[:],
                                ones_t[64:65, 0:64],
                                zt[64:65, :],
                                start=True,
                                stop=True,
                            )
                            zbs = apool.tile([64, 512], f32, tag="zbs")
                            nc.vector.tensor_copy(zbs[:], zb[:])
                            nc.vector.tensor_tensor(ctxn[:, h, qs], ctxps[0:64, :], zbs[:], mult)
                            nc.vector.tensor_scalar_add(
                                ctxn[:, h, qs], ctxn[:, h, qs], bv_t[:, h : h + 1]
                            )

                # ---- Phase 4: partial output projection ----
                with (
                    tc.tile_pool(name="wop", bufs=2) as wopool,
                    tc.tile_pool(name="ops", bufs=3, space="PSUM") as opsum,
                ):
                    Wore = Wo_d.rearrange("(h i) n -> i h n", i=HD)
                    for dh in range(2):
                        wo_t = wopool.tile([HD, HPC, 512], mdt, tag="wo")
                        nc.sync.dma_start(out=wo_t[:], in_=Wore[:, :, dh * 512 : (dh + 1) * 512])
                        for qt in range(NKT):
                            ps = opsum.tile([P, 512], f32, tag="op")
                            for h in range(HPC):
                                nc.tensor.matmul(
                                    ps[:],
                                    ctxn[:, h, qt * P : (qt + 1) * P],
                                    wo_t[:, h, :],
                                    start=(h == 0),
                                    stop=(h == HPC - 1),
                                )
                            ot = wopool.tile([P, 512], f32, tag="ot")
                            nc.vector.tensor_copy(ot[:], ps[:])
                            nc.sync.dma_start(
                                out=out_d[qt * P : (qt + 1) * P, dh * 512 : (dh + 1) * 512],
                                in_=ot[:],
                            )

    nc.compile()
    return nc


def kernel(X, mask, Wq, bq, Wk, bk, Wv, bv, Wo, bo):
    from concourse import bass_utils

    if "nc" not in _cache:
        _cache["nc"] = _build()
    nc = _cache["nc"]

    X = np.asarray(X, np.float32)
    mask = np.asarray(mask, np.float32)
    Wq, Wk, Wv, Wo = (np.asarray(a, np.float32) for a in (Wq, Wk, Wv, Wo))
    bq, bk, bv, bo = (np.asarray(a, np.float32) for a in (bq, bk, bv, bo))

    in_maps = []
    for c in range(NCORES):
        b, hs = divmod(c, 2)
        off = hs * CW
        in_maps.append(
            {
                "X": np.ascontiguousarray(X[b]),
                "mask": np.ascontiguousarray(mask[b]),
                "Wq": np.ascontiguousarray(Wq[:, off : off + CW]),
                "Wk": np.ascontiguousarray(Wk[:, off : off + CW]),
                "Wv": np.ascontiguousarray(Wv[:, off : off + CW]),
                "bq": np.ascontiguousarray(bq[off : off + CW]),
                "bk": np.ascontiguousarray(bk[off : off + CW]),
                "bv": np.ascontiguousarray(bv[off : off + CW]),
                "Wo": np.ascontiguousarray(Wo[off : off + CW, :]),
            }
        )

    trace = os.environ.get("KERNEL_TRACE", "0") == "1"
    res = bass_utils.run_bass_kernel_spmd(nc, in_maps, list(range(NCORES)), trace=trace)
    _cache["last_results"] = res

    parts = [res.results[c]["out"] for c in range(NCORES)]
    out = np.stack([parts[2 * b] + parts[2 * b + 1] for b in range(B)]) + bo
    return np.ascontiguousarray(out.astype(np.float32))


# revision 4
# speedup vs baseline: 1.8044x; 1.8044x over previous
"""Trainium2 Bass kernel: multi-head attention (B=4, S=2048, D=1024, H=16, HD=64).

Sharding: 8 cores = 4 batches x 2 head-groups. Core c handles batch c//2,
heads (c%2)*8 .. +8. Each core computes a partial output projection
out_partial[b] = ctx(heads) @ Wo[head_rows]; host sums the two partials per
batch and adds bo.

On-core layout ("k-major"): logits are computed transposed, LT[k, q], so the
softmax sum over keys is a partition-dim reduction done on the PE (fused into
the ctx matmul via an extra all-(mask)ones column appended to V), and the
attention-weighted sum ctxT[hd, q] = V'.T @ exp(LT) comes out in exactly the
layout the output projection needs as its stationary operand. No transposes of
the S x S matrix are ever needed. Softmax max-subtraction is skipped: logits
are ~N(0,1) here (X ~ N(0,1), W ~ N(0,1)/sqrt(D)), exp is safe in fp32, and
softmax is shift-invariant so the result matches the reference.

The additive -1e6 mask penalty is implemented exactly (for binary masks) by
zeroing masked keys' columns of V and the ones-column: exp(x - 1e6) underflows
to 0.0 in fp32 in the reference too, so weights and normalizer agree.

Matmul operands are bf16 (1 PE row/cycle; fp32 is 4, float32r measured ~2).
Accumulation is fp32 in PSUM, and the softmax normalizer Z stays in
fp32/float32r end-to-end. The per-query 1/Z is applied after broadcasting Z to
64 partitions with a rank-1 PE matmul (DVE ops on 1-partition rows are
lane-serial and cost ~3.4us, so the reciprocal runs on the broadcast tile).
"""

import os
import sys

import numpy as np

sys.path.insert(0, "/opt/trn_rl_repo")

B, S, D = 4, 2048, 1024
H, HD = 16, 64
NCORES = 8
HPC = H // 2  # heads per core
CW = HPC * HD  # per-core head-channel width (512)
P = 128
NKT = S // P  # 16 key tiles of 128
PIPE = 4  # logits matmul runs this many k-chunks ahead of the ctx matmul

_cache = {}


def _build():
    from concourse import bacc, masks, mybir, tile

    dt = mybir.dt
    f32 = dt.float32
    f32r = dt.float32r
    bf16 = dt.bfloat16
    Exp = mybir.ActivationFunctionType.Exp
    mult = mybir.AluOpType.mult

    nc = bacc.Bacc("TRN2", debug=False, target_bir_lowering=False, num_devices=NCORES)

    X_d = nc.dram_tensor("X", [S, D], f32, kind="ExternalInput").ap()
    mask_d = nc.dram_tensor("mask", [S], f32, kind="ExternalInput").ap()
    Wq_d = nc.dram_tensor("Wq", [D, CW], bf16, kind="ExternalInput").ap()
    Wk_d = nc.dram_tensor("Wk", [D, CW], bf16, kind="ExternalInput").ap()
    Wv_d = nc.dram_tensor("Wv", [D, CW], bf16, kind="ExternalInput").ap()
    bq_d = nc.dram_tensor("bq", [CW], f32, kind="ExternalInput").ap()
    bk_d = nc.dram_tensor("bk", [CW], f32, kind="ExternalInput").ap()
    bv_d = nc.dram_tensor("bv", [CW], f32, kind="ExternalInput").ap()
    Wo_d = nc.dram_tensor("Wo", [CW, D], bf16, kind="ExternalInput").ap()
    out_d = nc.dram_tensor("out", [S, D], f32, kind="ExternalOutput").ap()

    with tile.TileContext(nc) as tc:
        with (
            tc.tile_pool(name="const", bufs=1) as cpool,
            tc.tile_pool(name="dst", bufs=1) as dstpool,
        ):
            ident = cpool.tile([P, P], f32, tag="ident")
            masks.make_identity(nc, ident[:])
            ones_f = cpool.tile([1, 64], f32, tag="ones_f")
            nc.gpsimd.memset(ones_f[:], 1.0)
            ones_t = cpool.tile([1, 64], f32r, tag="ones")
            nc.vector.tensor_copy(ones_t[:], ones_f[:])
            ones8 = cpool.tile([P, HPC, 1], f32, tag="ones8")
            nc.gpsimd.memset(ones8[:], 1.0)
            mask_t = cpool.tile([P, NKT], f32, tag="maskt")
            nc.gpsimd.dma_start(out=mask_t[:], in_=mask_d.rearrange("(kt i) -> i kt", i=P))
            bq_t = cpool.tile([P, 4], f32, tag="bqt")
            nc.gpsimd.dma_start(out=bq_t[:], in_=bq_d.rearrange("(p i) -> i p", i=P))
            bk_t = cpool.tile([P, 4], f32, tag="bkt")
            nc.gpsimd.dma_start(out=bk_t[:], in_=bk_d.rearrange("(p i) -> i p", i=P))
            bv_t = cpool.tile([P, 4], f32, tag="bvt")
            nc.gpsimd.dma_start(out=bv_t[:], in_=bv_d.rearrange("(p i) -> i p", i=P))

            # QT/KT: [d-channel packs of 128 (2 heads), seq]; V': [k, kt, head, HD+1]
            QT = dstpool.tile([P, 4, S], bf16, tag="QT")
            KT = dstpool.tile([P, 4, S], bf16, tag="KT")
            Vt = dstpool.tile([P, NKT, HPC, HD + 1], bf16, tag="V")
            # normalized ctx^T, packed 2 heads per 128 partitions
            ctxn = dstpool.tile([P, 4, S], bf16, tag="ctxn")

            # ---- Phase 1+2: X transpose + Q/K/V projections (two seq-halves) ----
            with (
                tc.tile_pool(name="xtp", bufs=2) as xtpool,
                tc.tile_pool(name="tps", bufs=3, space="PSUM") as tpsum,
                tc.tile_pool(name="qps", bufs=3, space="PSUM") as qpsum,
            ):
                for half in range(2):
                    XTh = xtpool.tile([P, 8, S // 2], bf16, tag="xt", bufs=1)
                    for q8 in range(8):
                        qt = half * 8 + q8
                        xin = xtpool.tile([P, D], f32, tag="xin")
                        nc.sync.dma_start(out=xin[:], in_=X_d[qt * P : (qt + 1) * P, :])
                        for dc in range(8):
                            tp = tpsum.tile([P, P], f32, tag="tp")
                            nc.tensor.transpose(tp[:], xin[:, dc * P : (dc + 1) * P], ident[:])
                            nc.vector.tensor_copy(XTh[:, dc, q8 * P : (q8 + 1) * P], tp[:])
                    for W_d, b_t, dst in ((Wq_d, bq_t, QT), (Wk_d, bk_t, KT)):
                        Wre = W_d.rearrange("(dc p) m -> p dc m", p=P)
                        for pack in range(4):
                            wt = xtpool.tile([P, 8, P], bf16, tag="wqk")
                            nc.sync.dma_start(out=wt[:], in_=Wre[:, :, pack * P : (pack + 1) * P])
                            for q2 in range(2):
                                ps = qpsum.tile([P, 512], f32, tag="qp")
                                for dc in range(8):
                                    nc.tensor.matmul(
                                        ps[:],
                                        wt[:, dc, :],
                                        XTh[:, dc, q2 * 512 : (q2 + 1) * 512],
                                        start=(dc == 0),
                                        stop=(dc == 7),
                                    )
                                lo = half * 1024 + q2 * 512
                                nc.vector.tensor_scalar_add(
                                    dst[:, pack, lo : lo + 512], ps[:], b_t[:, pack : pack + 1]
                                )
                    vwt = xtpool.tile([P, 8, 512], bf16, tag="wv", bufs=1)
                    nc.sync.dma_start(out=vwt[:], in_=Wv_d.rearrange("(dc p) m -> p dc m", p=P))
                    for k8 in range(8):
                        kt = half * 8 + k8
                        ps = qpsum.tile([P, 512], f32, tag="qp")
                        for dc in range(8):
                            nc.tensor.matmul(
                                ps[:],
                                XTh[:, dc, k8 * P : (k8 + 1) * P],
                                vwt[:, dc, :],
                                start=(dc == 0),
                                stop=(dc == 7),
                            )
                        # masked V (bv folded into ctx later) + mask column for Z
                        nc.vector.tensor_scalar_mul(
                            Vt[:, kt, :, 0:HD],
                            ps.rearrange("p (h e) -> p h e", e=HD),
                            mask_t[:, kt : kt + 1],
                        )
                        nc.vector.tensor_scalar_mul(
                            Vt[:, kt, :, HD : HD + 1], ones8[:], mask_t[:, kt : kt + 1]
                        )

            # ---- Phase 3: attention per (head, 512-query tile) ----
            with (
                tc.tile_pool(name="attn", bufs=3) as apool,
                tc.tile_pool(name="lps", bufs=5, space="PSUM") as lpsum,
                tc.tile_pool(name="cps", bufs=2, space="PSUM") as cpsum,
                tc.tile_pool(name="zps", bufs=1, space="PSUM") as zpsum,
            ):
                for h in range(HPC):
                    hb = (h % 2) * 64
                    pk = h // 2
                    for q4 in range(4):
                        qs = slice(q4 * 512, (q4 + 1) * 512)
                        ctxps = cpsum.tile([HD + 1, 512], f32, tag="ctx")
                        ets = []
                        # software-pipelined: logits MM runs PIPE ahead of ctx MM
                        for kc in range(NKT + PIPE):
                            if kc < NKT:
                                lps = lpsum.tile([P, 512], f32, tag="lg")
                                nc.tensor.matmul(
                                    lps[:],
                                    KT[hb : hb + 64, pk, kc * P : (kc + 1) * P],
                                    QT[hb : hb + 64, pk, qs],
                                    start=True,
                                    stop=True,
                                )
                                et = apool.tile([P, 512], bf16, tag="exp", bufs=PIPE + 2)
                                nc.scalar.activation(et[:], lps[:], Exp, scale=0.125)
                                ets.append(et)
                            if kc >= PIPE:
                                kd = kc - PIPE
                                nc.tensor.matmul(
                                    ctxps[:],
                                    Vt[:, kd, h, :],
                                    ets[kd][:],
                                    start=(kd == 0),
                                    stop=(kd == NKT - 1),
                                )
                        # Z row -> SBUF (ACT; DVE 1-partition ops are lane-serial)
                        zrow = apool.tile([1, 512], f32r, tag="zrow")
                        nc.scalar.copy(zrow[:], ctxps[64:65, :])
                        # broadcast Z to 64 partitions via rank-1 matmul, then 1/Z
                        zb = zpsum.tile([64, 512], f32, tag="zb")
                        nc.tensor.matmul(zb[:], ones_t[:], zrow[:], start=True, stop=True)
                        zbs = apool.tile([64, 512], f32, tag="zbs")
                        nc.vector.reciprocal(zbs[:], zb[:])
                        dst = ctxn[hb : hb + 64, pk, qs]
                        nc.vector.tensor_tensor(dst, ctxps[0:64, :], zbs[:], mult)
                        nc.vector.tensor_scalar_add(dst, dst, bv_t[hb : hb + 64, pk : pk + 1])

            # ---- Phase 4: partial output projection ----
            with (
                tc.tile_pool(name="wop", bufs=2) as wopool,
                tc.tile_pool(name="ops", bufs=3, space="PSUM") as opsum,
            ):
                Wore = Wo_d.rearrange("(p i) n -> i p n", i=P)
                for dh in range(2):
                    wo_t = wopool.tile([P, 4, 512], bf16, tag="wo")
                    nc.sync.dma_start(out=wo_t[:], in_=Wore[:, :, dh * 512 : (dh + 1) * 512])
                    for qt in range(NKT):
                        ps = opsum.tile([P, 512], f32, tag="op")
                        for pk in range(4):
                            nc.tensor.matmul(
                                ps[:],
                                ctxn[:, pk, qt * P : (qt + 1) * P],
                                wo_t[:, pk, :],
                                start=(pk == 0),
                                stop=(pk == 3),
                            )
                        ot = wopool.tile([P, 512], f32, tag="ot")
                        nc.vector.tensor_copy(ot[:], ps[:])
                        nc.sync.dma_start(
                            out=out_d[qt * P : (qt + 1) * P, dh * 512 : (dh + 1) * 512],
                            in_=ot[:],
                        )

    nc.compile()
    return nc


def kernel(X, mask, Wq, bq, Wk, bk, Wv, bv, Wo, bo):
    import ml_dtypes

    from concourse import bass_utils

    if "nc" not in _cache:
        _cache["nc"] = _build()
    nc = _cache["nc"]

    bfnp = ml_dtypes.bfloat16
    X = np.asarray(X, np.float32)
    mask = np.asarray(mask, np.float32)
    Wq, Wk, Wv, Wo = (np.asarray(a, np.float32) for a in (Wq, Wk, Wv, Wo))
    bq, bk, bv, bo = (np.asarray(a, np.float32) for a in (bq, bk, bv, bo))

    in_maps = []
    for c in range(NCORES):
        b, hs = divmod(c, 2)
        off = hs * CW
        in_maps.append(
            {
                "X": np.ascontiguousarray(X[b]),
                "mask": np.ascontiguousarray(mask[b]),
                "Wq": np.ascontiguousarray(Wq[:, off : off + CW]).astype(bfnp),
                "Wk": np.ascontiguousarray(Wk[:, off : off + CW]).astype(bfnp),
                "Wv": np.ascontiguousarray(Wv[:, off : off + CW]).astype(bfnp),
                "bq": np.ascontiguousarray(bq[off : off + CW]),
                "bk": np.ascontiguousarray(bk[off : off + CW]),
                "bv": np.ascontiguousarray(bv[off : off + CW]),
                "Wo": np.ascontiguousarray(Wo[off : off + CW, :]).astype(bfnp),
            }
        )

    trace = os.environ.get("KERNEL_TRACE", "0") == "1"
    res = bass_utils.run_bass_kernel_spmd(nc, in_maps, list(range(NCORES)), trace=trace)
    _cache["last_results"] = res

    parts = [res.results[c]["out"] for c in range(NCORES)]
    out = np.stack([parts[2 * b] + parts[2 * b + 1] for b in range(B)]) + bo
    return np.ascontiguousarray(out.astype(np.float32))
